# revision 18
# baseline (speedup 1.0000x reference)
"""Trainium2 Bass kernel for a single AttnDecoderRNN decode step (batch=1).

Distribution over 8 NeuronCores (SPMD, one program, per-core input values):
  - emb, out_W, out_b: sharded over vocab (6400 rows/core, padded 50257->51200).
  - Embedding row gather: indirect DMA from the vocab-sharded table (the
    owning core gathers the token row, the others gather an all-zero pad
    row), then AllReduce-add to broadcast the embedded vector to all cores.
  - Attention (L=128): replicated on every core (weights are tiny).
  - combine layer: output-sharded; core m computes x[m*128:(m+1)*128].
  - LSTM gates: column-sharded partial sums (core m contracts only its
    128-slice of x and of h0) + AllReduce over the [4096] partial gates;
    the elementwise LSTM cell is then computed replicated, so every core
    has the full h_new without another collective.
  - Output projection: vocab-on-partitions matvec in bf16 (halves the HBM
    traffic of the dominant 206MB weight), sharded logsumexp with a final
    AllReduce of per-partition exp-sums, log-softmax subtract on device.

The host side only slices / transposes / pads inputs into the per-core DMA
layouts, and reassembles the per-core output shards.
"""

import numpy as np
import ml_dtypes

H = 1024          # hidden size
L = 128           # attention length
V = 50257         # vocab
NCORES = 8
VS = 6400         # vocab shard per core (50 tiles of 128), 8*6400 = 51200
VT = VS // 128    # 50 vocab tiles per core
NEG_BIG = -1.0e30
NKC = H // 128    # 8 contraction chunks of 128 for H-sized dims

_BF16 = ml_dtypes.bfloat16
# attn/enc/comb/LSTM weights in bf16 (fp32 accumulate): saves ~10us of DMA
# but raises h_new/attn_weights error from ~1e-6 to ~3e-3 scale-relative.
# Kept off: the kernel already beats the memory-roofline target comfortably
# and the fp32 stage path keeps all non-logit outputs at fp32 exactness.
STAGE_BF16 = False

_prog_cache = {}


# ---------------------------------------------------------------------------
# device program
# ---------------------------------------------------------------------------

def _build_program(repeat=1):
    import concourse.bass as bass
    import concourse.mybir as mybir
    import concourse.tile as tile
    from concourse import bacc

    dt = mybir.dt
    f32 = dt.float32
    bf16 = dt.bfloat16
    sdt = bf16 if STAGE_BF16 else f32

    nc = bacc.Bacc("TRN2", target_bir_lowering=False, debug=False,
                   num_devices=NCORES)

    # ---- external inputs (per-core values supplied by the host) ----
    # emb shard has one extra all-zero row; non-owner cores gather that row
    # so the AllReduce-add broadcast needs no masking.
    emb_sh = nc.dram_tensor("emb_sh", [VS + 1, H], f32, kind="ExternalInput")
    gather_offs = nc.dram_tensor("gather_offs", [128, 1], dt.int32,
                                 kind="ExternalInput")
    enc = nc.dram_tensor("enc", [L, H], sdt, kind="ExternalInput")
    attn_we = nc.dram_tensor("attn_we", [128, NKC * 128], sdt,
                             kind="ExternalInput")
    attn_wh = nc.dram_tensor("attn_wh", [128, NKC * 128], sdt,
                             kind="ExternalInput")
    attn_b_col = nc.dram_tensor("attn_b_col", [128, 1], f32,
                                kind="ExternalInput")
    comb_we = nc.dram_tensor("comb_we", [128, NKC * 128], sdt,
                             kind="ExternalInput")
    comb_wa = nc.dram_tensor("comb_wa", [128, NKC * 128], sdt,
                             kind="ExternalInput")
    comb_b_col = nc.dram_tensor("comb_b_col", [128, 1], f32,
                                kind="ExternalInput")
    wih_sh = nc.dram_tensor("wih_sh", [128, 32 * 128], sdt,
                            kind="ExternalInput")
    whh_sh = nc.dram_tensor("whh_sh", [128, 32 * 128], sdt,
                            kind="ExternalInput")
    bias_g_col = nc.dram_tensor("bias_g_col", [128, 32], f32,
                                kind="ExternalInput")
    h0_col = nc.dram_tensor("h0_col", [128, NKC], sdt, kind="ExternalInput")
    c0_col = nc.dram_tensor("c0_col", [128, NKC], f32, kind="ExternalInput")
    h0_my = nc.dram_tensor("h0_my", [128, 1], sdt, kind="ExternalInput")
    # out-projection weights, bf16, 5 blocks of 10 vocab tiles each
    NBLK = 5
    TPB = VT // NBLK  # vocab tiles per block
    outw = [
        nc.dram_tensor(f"outw{b}", [128, TPB * 1024], bf16,
                       kind="ExternalInput")
        for b in range(NBLK)
    ]
    out_b_col = nc.dram_tensor("out_b_col", [128, VT], f32,
                               kind="ExternalInput")

    # ---- external outputs ----
    out_logprob = nc.dram_tensor("out_logprob", [128, VT], f32,
                                 kind="ExternalOutput")
    out_h = nc.dram_tensor("out_h", [128, NKC], f32, kind="ExternalOutput")
    out_c = nc.dram_tensor("out_c", [128, NKC], f32, kind="ExternalOutput")
    out_attw = nc.dram_tensor("out_attw", [128, 1], f32,
                              kind="ExternalOutput")

    RG = [list(range(NCORES))]

    with tile.TileContext(nc) as tc:
        with (
            tc.tile_pool(name="w", bufs=1) as wp,
            tc.tile_pool(name="sb", bufs=1) as sbp,
            tc.tile_pool(name="ps", bufs=2, space="PSUM") as psp,
            tc.tile_pool(name="dram", bufs=1, space="DRAM") as dram,
        ):
            for it in range(repeat):
                _build_body(nc, tc, wp, sbp, psp, dram, mybir, bass,
                            emb_sh, gather_offs, enc,
                            attn_we, attn_wh, attn_b_col,
                            comb_we, comb_wa, comb_b_col,
                            wih_sh, whh_sh, bias_g_col,
                            h0_col, c0_col, h0_my,
                            outw, out_b_col,
                            out_logprob, out_h, out_c, out_attw,
                            RG, NBLK, TPB, it)

    nc.compile()
    return nc


def _build_body(nc, tc, wp, sbp, psp, dram, mybir, bass,
                emb_sh, gather_offs, enc,
                attn_we, attn_wh, attn_b_col,
                comb_we, comb_wa, comb_b_col,
                wih_sh, whh_sh, bias_g_col,
                h0_col, c0_col, h0_my,
                outw, out_b_col,
                out_logprob, out_h, out_c, out_attw,
                RG, NBLK, TPB, it):
    dt = mybir.dt
    f32 = dt.float32
    bf16 = dt.bfloat16
    sdt = bf16 if STAGE_BF16 else f32
    AF = mybir.ActivationFunctionType
    OP = mybir.AluOpType

    # Tags are shared across repeat iterations: pool slots (and SBUF space)
    # are reused, and iteration i+1's loads serialize behind iteration i's
    # last reads, giving honest steady-state per-iteration timing.
    t = lambda name: name

    # ---- embedding gather + broadcast (AllReduce #1) -- issued FIRST so its
    # tiny DMAs and the collective are not queued behind the weight streams.
    # emb_sh [VS+1, H] viewed as [(VS+1)*128, 8]; row r = v*128 + q holds
    # emb_sh[v, q*8:(q+1)*8].  Gathered tile: g[p, j] = embedded[p*8 + j]
    # (a fixed permutation of the 1024 dims; the attn_we/comb_we host
    # layouts use the same permutation).  Non-owner cores gather the
    # all-zero row VS, so the AllReduce-add is an unmasked broadcast.
    offs_sb = wp.tile([128, 1], dt.int32, tag=t("offs"))
    nc.gpsimd.dma_start(offs_sb[:], gather_offs[:])
    emb_view = emb_sh.ap().rearrange("v (q e) -> (v q) e", e=8)
    gath = sbp.tile([128, 8], f32, tag=t("gath"))
    nc.gpsimd.indirect_dma_start(
        out=gath[:],
        out_offset=None,
        in_=emb_view,
        in_offset=bass.IndirectOffsetOnAxis(ap=offs_sb[:, :1], axis=0),
    )
    emb_bin = dram.tile([128, 8], f32, tag=t("emb_bin"))
    nc.gpsimd.dma_start(emb_bin[:], gath[:])
    emb_bout = dram.tile([128, 8], f32, tag=t("emb_bout"))
    nc.gpsimd.collective_compute(
        "AllReduce", OP.add, replica_groups=RG,
        ins=[emb_bin.opt()], outs=[emb_bout.opt()],
    )
    embP_f = sbp.tile([128, 8], f32, tag=t("embP_f"))
    nc.gpsimd.dma_start(embP_f[:], emb_bout[:])
    embP = sbp.tile([128, 8], sdt, tag=t("embP"))
    nc.vector.tensor_copy(embP[:], embP_f[:])

    # ---- load small tensors / weights into SBUF ----
    h0c_sb = wp.tile([128, NKC], sdt, tag=t("h0c"))
    nc.sync.dma_start(h0c_sb[:], h0_col[:])
    c0c_sb = wp.tile([128, NKC], f32, tag=t("c0c"))
    nc.sync.dma_start(c0c_sb[:], c0_col[:])
    h0my_sb = wp.tile([128, 1], sdt, tag=t("h0my"))
    nc.sync.dma_start(h0my_sb[:], h0_my[:])
    attnb_sb = wp.tile([128, 1], f32, tag=t("attnb"))
    nc.sync.dma_start(attnb_sb[:], attn_b_col[:])
    combb_sb = wp.tile([128, 1], f32, tag=t("combb"))
    nc.sync.dma_start(combb_sb[:], comb_b_col[:])
    biasg_sb = wp.tile([128, 32], f32, tag=t("biasg"))
    nc.sync.dma_start(biasg_sb[:], bias_g_col[:])
    outb_sb = wp.tile([128, VT], f32, tag=t("outb"))
    nc.sync.dma_start(outb_sb[:], out_b_col[:])

    awe_sb = wp.tile([128, NKC * 128], sdt, tag=t("awe"))
    nc.sync.dma_start(awe_sb[:], attn_we[:])
    awh_sb = wp.tile([128, NKC * 128], sdt, tag=t("awh"))
    nc.sync.dma_start(awh_sb[:], attn_wh[:])
    enc_sb = wp.tile([L, H], sdt, tag=t("enc"))
    nc.sync.dma_start(enc_sb[:], enc[:])
    cwe_sb = wp.tile([128, NKC * 128], sdt, tag=t("cwe"))
    nc.sync.dma_start(cwe_sb[:], comb_we[:])
    cwa_sb = wp.tile([128, NKC * 128], sdt, tag=t("cwa"))
    nc.sync.dma_start(cwa_sb[:], comb_wa[:])
    wih_sb = wp.tile([128, 32 * 128], sdt, tag=t("wih"))
    nc.sync.dma_start(wih_sb[:], wih_sh[:])
    whh_sb = wp.tile([128, 32 * 128], sdt, tag=t("whh"))
    nc.sync.dma_start(whh_sb[:], whh_sh[:])

    # outw block DMAs are emitted last among the loads: they are consumed
    # last (after h_new) and must not delay the small stage weights.
    outw_sb = []
    for b in range(NBLK):
        ow = wp.tile([128, TPB * 1024], bf16, tag=t(f"outw{b}"))
        nc.sync.dma_start(ow[:], outw[b][:])
        outw_sb.append(ow)

    ones_col = wp.tile([128, 1], f32, tag=t("ones"))
    nc.vector.memset(ones_col[:], 1.0)
    ones_row = wp.tile([1, 128], f32, tag=t("ones_row"))
    nc.vector.memset(ones_row[:], 1.0)

    # ---- attention scores (all 128, replicated) ----
    ps_s = psp.tile([128, 1], f32, tag=t("ps_small"), bufs=2)
    for c in range(NKC):
        nc.tensor.matmul(ps_s[:], awh_sb[:, c * 128:(c + 1) * 128],
                         h0c_sb[:, c:c + 1], start=(c == 0), stop=False)
    for c in range(NKC):
        nc.tensor.matmul(ps_s[:], awe_sb[:, c * 128:(c + 1) * 128],
                         embP[:, c:c + 1], start=False, stop=(c == NKC - 1))
    # softmax over partitions; scores are O(1), exp without max-shift is safe
    exp_s = sbp.tile([128, 1], f32, tag=t("exp_s"))
    nc.scalar.activation(exp_s[:], ps_s[:], AF.Exp, bias=attnb_sb[:, :1])
    ps_ssum = psp.tile([1, 1], f32, tag=t("ps_small"), bufs=2)
    nc.tensor.matmul(ps_ssum[:], exp_s[:], ones_col[:], start=True, stop=True)
    ssum_sb = sbp.tile([1, 1], f32, tag=t("ssum"))
    nc.vector.tensor_copy(ssum_sb[:], ps_ssum[:])
    rinv_sb = sbp.tile([1, 1], f32, tag=t("rinv"))
    nc.vector.reciprocal(rinv_sb[:], ssum_sb[:])
    # broadcast 1/sum to all partitions via PE (ones_row.T @ rinv)
    ps_rb = psp.tile([128, 1], f32, tag=t("ps_small"), bufs=2)
    nc.tensor.matmul(ps_rb[:], ones_row[:], rinv_sb[:], start=True, stop=True)
    rb_sb = sbp.tile([128, 1], f32, tag=t("rb"))
    nc.vector.tensor_copy(rb_sb[:], ps_rb[:])
    attw = sbp.tile([128, 1], f32, tag=t("attw"))
    nc.vector.tensor_mul(attw[:], exp_s[:], rb_sb[:])
    nc.scalar.dma_start(out_attw[:], attw[:])
    attw_s = sbp.tile([128, 1], sdt, tag=t("attw_s"))
    nc.vector.tensor_copy(attw_s[:], attw[:])

    # ---- attn_applied, directly in column-chunk form [128, 8] ----
    ps_aa = psp.tile([128, 8], f32, tag=t("ps_small"), bufs=2)
    for c in range(NKC):
        nc.tensor.matmul(ps_aa[:, c:c + 1], enc_sb[:, c * 128:(c + 1) * 128],
                         attw_s[:], start=True, stop=True)
    aa_sb = sbp.tile([128, 8], sdt, tag=t("aa"))
    nc.vector.tensor_copy(aa_sb[:], ps_aa[:])

    # ---- combine + relu: x slice for this core ----
    ps_x = psp.tile([128, 1], f32, tag=t("ps_small"), bufs=2)
    for c in range(NKC):
        nc.tensor.matmul(ps_x[:], cwe_sb[:, c * 128:(c + 1) * 128],
                         embP[:, c:c + 1], start=(c == 0), stop=False)
    for c in range(NKC):
        nc.tensor.matmul(ps_x[:], cwa_sb[:, c * 128:(c + 1) * 128],
                         aa_sb[:, c:c + 1], start=False, stop=(c == NKC - 1))
    x_col = sbp.tile([128, 1], sdt, tag=t("x_col"))
    nc.scalar.activation(x_col[:], ps_x[:], AF.Relu, bias=combb_sb[:, :1])

    # ---- partial LSTM gates (column-sharded) + AllReduce #2 ----
    ps_g = psp.tile([128, 32], f32, tag=t("ps_g"), bufs=1)
    for tau in range(32):
        sl = slice(tau * 128, (tau + 1) * 128)
        nc.tensor.matmul(ps_g[:, tau:tau + 1], wih_sb[:, sl], x_col[:],
                         start=True, stop=False)
        nc.tensor.matmul(ps_g[:, tau:tau + 1], whh_sb[:, sl], h0my_sb[:],
                         start=False, stop=True)
    gpart = sbp.tile([128, 32], f32, tag=t("gpart"))
    nc.vector.tensor_copy(gpart[:], ps_g[:])
    g_bin = dram.tile([128, 32], f32, tag=t("g_bin"))
    nc.gpsimd.dma_start(g_bin[:], gpart[:])
    g_bout = dram.tile([128, 32], f32, tag=t("g_bout"))
    nc.gpsimd.collective_compute(
        "AllReduce", OP.add, replica_groups=RG,
        ins=[g_bin.opt()], outs=[g_bout.opt()],
    )
    gsum = sbp.tile([128, 32], f32, tag=t("gsum"))
    nc.gpsimd.dma_start(gsum[:], g_bout[:])

    # ---- LSTM cell elementwise (replicated) ----
    gf = sbp.tile([128, 32], f32, tag=t("gf"))
    nc.vector.tensor_add(gf[:], gsum[:], biasg_sb[:])
    sig_i = sbp.tile([128, 8], f32, tag=t("sig_i"))
    nc.scalar.activation(sig_i[:], gf[:, 0:8], AF.Sigmoid)
    sig_f = sbp.tile([128, 8], f32, tag=t("sig_f"))
    nc.scalar.activation(sig_f[:], gf[:, 8:16], AF.Sigmoid)
    tanh_g = sbp.tile([128, 8], f32, tag=t("tanh_g"))
    nc.scalar.activation(tanh_g[:], gf[:, 16:24], AF.Tanh)
    sig_o = sbp.tile([128, 8], f32, tag=t("sig_o"))
    nc.scalar.activation(sig_o[:], gf[:, 24:32], AF.Sigmoid)
    t1 = sbp.tile([128, 8], f32, tag=t("t1"))
    nc.vector.tensor_mul(t1[:], sig_f[:], c0c_sb[:])
    t2 = sbp.tile([128, 8], f32, tag=t("t2"))
    nc.vector.tensor_mul(t2[:], sig_i[:], tanh_g[:])
    c_new = sbp.tile([128, 8], f32, tag=t("c_new"))
    nc.vector.tensor_add(c_new[:], t1[:], t2[:])
    tanh_c = sbp.tile([128, 8], f32, tag=t("tanh_c"))
    nc.scalar.activation(tanh_c[:], c_new[:], AF.Tanh)
    h_new = sbp.tile([128, 8], f32, tag=t("h_new"))
    nc.vector.tensor_mul(h_new[:], sig_o[:], tanh_c[:])
    nc.scalar.dma_start(out_h[:], h_new[:])
    nc.scalar.dma_start(out_c[:], c_new[:])
    h_bf = sbp.tile([128, 8], bf16, tag=t("h_bf"))
    nc.vector.tensor_copy(h_bf[:], h_new[:])

    # ---- output projection matvec (vocab on partitions, bf16) ----
    logits = sbp.tile([128, VT], f32, tag=t("logits"))
    for v in range(VT):
        blk, off = divmod(v, TPB)
        base = off * 1024
        po = psp.tile([128, 1], f32, tag=t("po"), bufs=4)
        for c in range(NKC):
            nc.tensor.matmul(po[:], outw_sb[blk][:, base + c * 128:
                                                 base + (c + 1) * 128],
                             h_bf[:, c:c + 1],
                             start=(c == 0), stop=(c == NKC - 1))
        nc.vector.tensor_add(logits[:, v:v + 1], po[:], outb_sb[:, v:v + 1])

    # ---- sharded logsumexp (AllReduce #3) + log-softmax ----
    exps = sbp.tile([128, VT], f32, tag=t("exps"))
    nc.scalar.activation(exps[:], logits[:], AF.Exp)
    se_col = sbp.tile([128, 1], f32, tag=t("se_col"))
    nc.vector.tensor_reduce(se_col[:], exps[:], axis=mybir.AxisListType.X,
                            op=mybir.AluOpType.add)
    se_bin = dram.tile([128, 1], f32, tag=t("se_bin"))
    nc.gpsimd.dma_start(se_bin[:], se_col[:])
    se_bout = dram.tile([128, 1], f32, tag=t("se_bout"))
    nc.gpsimd.collective_compute(
        "AllReduce", OP.add, replica_groups=RG,
        ins=[se_bin.opt()], outs=[se_bout.opt()],
    )
    se_g = sbp.tile([128, 1], f32, tag=t("se_g"))
    nc.gpsimd.dma_start(se_g[:], se_bout[:])
    ps_tot = psp.tile([1, 1], f32, tag=t("ps_small"), bufs=2)
    nc.tensor.matmul(ps_tot[:], se_g[:], ones_col[:], start=True, stop=True)
    tot_sb = sbp.tile([1, 1], f32, tag=t("tot"))
    nc.vector.tensor_copy(tot_sb[:], ps_tot[:])
    lnz = sbp.tile([1, 1], f32, tag=t("lnz"))
    nc.scalar.activation(lnz[:], tot_sb[:], AF.Ln)
    nlz = sbp.tile([1, 1], f32, tag=t("nlz"))
    nc.vector.tensor_scalar_mul(nlz[:], lnz[:], -1.0)
    # broadcast -logZ to all partitions via PE, then subtract
    ps_nlzb = psp.tile([128, 1], f32, tag=t("ps_small"), bufs=2)
    nc.tensor.matmul(ps_nlzb[:], ones_row[:], nlz[:], start=True, stop=True)
    nlzb_sb = sbp.tile([128, 1], f32, tag=t("nlzb"))
    nc.vector.tensor_copy(nlzb_sb[:], ps_nlzb[:])
    out_lp = sbp.tile([128, VT], f32, tag=t("out_lp"))
    nc.vector.tensor_add(out_lp[:], logits[:],
                         nlzb_sb[:, :1].to_broadcast([128, VT]))
    nc.scalar.dma_start(out_logprob[:], out_lp[:])


# ---------------------------------------------------------------------------
# host-side sharding
# ---------------------------------------------------------------------------

def _shard_inputs(tokens, h0, c0, encoder_outputs, emb, attn_W, attn_b,
                  comb_W, comb_b, W_ih, W_hh, b_ih, b_hh, out_W, out_b):
    f32 = np.float32
    tokens = np.asarray(tokens)
    h0 = np.asarray(h0, f32).reshape(H)
    c0 = np.asarray(c0, f32).reshape(H)
    enc = np.ascontiguousarray(np.asarray(encoder_outputs, f32))
    emb = np.asarray(emb, f32)
    attn_W = np.asarray(attn_W, f32)
    attn_b = np.asarray(attn_b, f32).reshape(L)
    comb_W = np.asarray(comb_W, f32)
    comb_b = np.asarray(comb_b, f32).reshape(H)
    W_ih = np.asarray(W_ih, f32)
    W_hh = np.asarray(W_hh, f32)
    b_ih = np.asarray(b_ih, f32).reshape(4 * H)
    b_hh = np.asarray(b_hh, f32).reshape(4 * H)
    out_W = np.asarray(out_W, f32)
    out_b = np.asarray(out_b, f32).reshape(V)

    tok = int(np.asarray(tokens).reshape(-1)[0])
    owner = tok // VS
    loc = tok - owner * VS

    # replicated pieces
    # attn_we[p, c*128+l] = attn_W[l, p*8+c]  (permuted embedded index p*8+c)
    A = attn_W[:, :H].reshape(L, 128, 8)            # [l][p][c]
    attn_we = np.ascontiguousarray(
        A.transpose(1, 2, 0).reshape(128, NKC * 128))
    B = attn_W[:, H:].reshape(L, 8, 128)            # [l][c][p]
    attn_wh = np.ascontiguousarray(
        B.transpose(2, 1, 0).reshape(128, NKC * 128))
    attn_b_col = np.ascontiguousarray(attn_b.reshape(128, 1))
    bias_g_col = np.ascontiguousarray((b_ih + b_hh).reshape(32, 128).T)
    h0_col = np.ascontiguousarray(h0.reshape(8, 128).T)
    c0_col = np.ascontiguousarray(c0.reshape(8, 128).T)

    # padded vocab-sharded tables
    emb_pad = np.zeros((NCORES * VS, H), f32)
    emb_pad[:V] = emb
    outw_pad = np.zeros((NCORES * VS, H), f32)
    outw_pad[:V] = out_W
    outb_pad = np.full(NCORES * VS, NEG_BIG, f32)
    outb_pad[:V] = out_b

    in_maps = []
    for m in range(NCORES):
        r0 = m * VS
        # comb: rows for this core's x slice
        C = comb_W[m * 128:(m + 1) * 128, :H].reshape(128, 128, 8)  # [j][p][c]
        comb_we = np.ascontiguousarray(
            C.transpose(1, 2, 0).reshape(128, NKC * 128))
        D = comb_W[m * 128:(m + 1) * 128, H:].reshape(128, 8, 128)  # [j][c][p]
        comb_wa = np.ascontiguousarray(
            D.transpose(2, 1, 0).reshape(128, NKC * 128))
        comb_b_col = np.ascontiguousarray(
            comb_b[m * 128:(m + 1) * 128].reshape(128, 1))
        # LSTM column shards: [k][tau*128+j] = W[tau*128+j, m*128+k]
        wih_sh = np.ascontiguousarray(W_ih[:, m * 128:(m + 1) * 128].T)
        whh_sh = np.ascontiguousarray(W_hh[:, m * 128:(m + 1) * 128].T)
        h0_my = np.ascontiguousarray(h0_col[:, m:m + 1])
        # out projection: outw[p, t*1024 + c*128 + j] = W[r0+t*128+j, c*128+p]
        S = outw_pad[r0:r0 + VS].reshape(VT, 128, 8, 128)  # [t][j][c][p]
        ow = np.ascontiguousarray(
            S.transpose(3, 0, 2, 1).reshape(128, VT * 1024)).astype(_BF16)
        out_b_col = np.ascontiguousarray(
            outb_pad[r0:r0 + VS].reshape(VT, 128).T)
        # embedding shard (+1 zero row) + gather offsets
        emb_shard = np.zeros((VS + 1, H), f32)
        emb_shard[:VS] = emb_pad[r0:r0 + VS]
        if m == owner:
            offs = (loc * 128 + np.arange(128, dtype=np.int32))
        else:
            offs = (VS * 128 + np.arange(128, dtype=np.int32))
        snp = _BF16 if STAGE_BF16 else f32
        im = {
            "emb_sh": emb_shard,
            "gather_offs": offs.reshape(128, 1),
            "enc": enc.astype(snp),
            "attn_we": attn_we.astype(snp),
            "attn_wh": attn_wh.astype(snp),
            "attn_b_col": attn_b_col,
            "comb_we": comb_we.astype(snp),
            "comb_wa": comb_wa.astype(snp),
            "comb_b_col": comb_b_col,
            "wih_sh": wih_sh.astype(snp),
            "whh_sh": whh_sh.astype(snp),
            "bias_g_col": bias_g_col,
            "h0_col": h0_col.astype(snp),
            "c0_col": c0_col,
            "h0_my": h0_my.astype(snp),
            "out_b_col": out_b_col,
        }
        NBLK = 5
        TPB = VT // NBLK
        for b in range(NBLK):
            im[f"outw{b}"] = np.ascontiguousarray(
                ow[:, b * TPB * 1024:(b + 1) * TPB * 1024])
        in_maps.append(im)
    return in_maps


def _assemble(results):
    logp = np.concatenate(
        [results[m]["out_logprob"].T.reshape(VS) for m in range(NCORES)])
    output = logp[:V].reshape(1, V).astype(np.float32)
    h_new = results[0]["out_h"].T.reshape(1, 1, H).astype(np.float32)
    c_new = results[0]["out_c"].T.reshape(1, 1, H).astype(np.float32)
    attw = results[0]["out_attw"].reshape(1, L).astype(np.float32)
    return output, h_new, c_new, attw


# ---------------------------------------------------------------------------
# public entry point
# ---------------------------------------------------------------------------

def _get_program(repeat=1):
    if repeat not in _prog_cache:
        _prog_cache[repeat] = _build_program(repeat)
    return _prog_cache[repeat]


_runner_cache = {}


def _make_runner(nc):
    """Persistent jitted runner for repeat calls (avoids re-jitting the
    NEFF custom call inside run_bass_kernel_spmd on every invocation)."""
    import jax
    from jax.sharding import Mesh, PartitionSpec, NamedSharding
    from jax.experimental.shard_map import shard_map
    from concourse import bass2jax, mybir

    bass2jax.install_neuronx_cc_hook()
    partition_name = (nc.partition_id_tensor.name
                      if nc.partition_id_tensor else None)
    in_names, out_names, out_avals, zero_shapes = [], [], [], []
    for alloc in nc.m.functions[0].allocations:
        if not isinstance(alloc, mybir.MemoryLocationSet):
            continue
        name = alloc.memorylocations[0].name
        if alloc.kind == "ExternalInput":
            if name != partition_name:
                in_names.append(name)
        elif alloc.kind == "ExternalOutput":
            out_names.append(name)
            shape = tuple(alloc.tensor_shape)
            dtype = mybir.dt.np(alloc.dtype)
            out_avals.append(jax.core.ShapedArray(shape, dtype))
            zero_shapes.append((shape, dtype))
    n_params = len(in_names)
    n_outs = len(out_names)
    all_names = in_names + out_names
    if partition_name is not None:
        all_names = all_names + [partition_name]

    def _body(*args):
        operands = list(args)
        if partition_name is not None:
            operands.append(bass2jax.partition_id_tensor())
        outs = bass2jax._bass_exec_p.bind(
            *operands,
            out_avals=tuple(out_avals),
            in_names=tuple(all_names),
            out_names=tuple(out_names),
            lowering_input_output_aliases=(),
            sim_require_finite=True,
            sim_require_nnan=True,
            nc=nc,
        )
        return tuple(outs)

    devices = jax.devices()[:NCORES]
    mesh = Mesh(np.asarray(devices), ("core",))
    in_specs = (PartitionSpec("core"),) * (n_params + n_outs)
    out_specs = (PartitionSpec("core"),) * n_outs
    donate = tuple(range(n_params, n_params + n_outs))
    fn = jax.jit(
        shard_map(_body, mesh=mesh, in_specs=in_specs, out_specs=out_specs,
                  check_rep=False),
        donate_argnums=donate, keep_unused=True,
    )
    sharding = NamedSharding(mesh, PartitionSpec("core"))

    def run(in_maps):
        concat_in = [
            jax.device_put(
                np.concatenate([np.asarray(in_maps[c][n])
                                for c in range(NCORES)], axis=0), sharding)
            for n in in_names
        ]
        zeros = [
            jax.device_put(np.zeros((NCORES * s[0], *s[1:]), d), sharding)
            for s, d in zero_shapes
        ]
        outs = fn(*concat_in, *zeros)
        jax.block_until_ready(outs)
        return [
            {name: np.asarray(outs[i]).reshape(NCORES, *out_avals[i].shape)[c]
             for i, name in enumerate(out_names)}
            for c in range(NCORES)
        ]

    return run


def kernel(**inputs):
    from concourse.bass_utils import run_bass_kernel_spmd

    nc = _get_program()
    in_maps = _shard_inputs(**inputs)
    if "run" not in _runner_cache:
        # first call: the sanctioned SPMD path (compiles the NEFF)
        res = run_bass_kernel_spmd(nc, in_maps, list(range(NCORES)))
        _runner_cache["run"] = _make_runner(nc)
        return _assemble(res.results)
    return _assemble(_runner_cache["run"](in_maps))


# revision 25
# speedup vs baseline: 1.5508x; 1.5508x over previous
"""Trainium2 Bass kernel for a single AttnDecoderRNN decode step (batch=1).

Distribution over 8 NeuronCores (SPMD, one program, per-core input values):
  - emb, out_W, out_b: sharded over vocab (6400 rows/core, padded 50257->51200).
  - Embedding row gather: indirect DMA from the vocab-sharded table (the
    owning core gathers the token row, the others gather an all-zero pad
    row), then AllReduce-add to broadcast the embedded vector to all cores.
  - Attention (L=128): replicated on every core (weights are tiny).
  - combine layer: output-sharded; core m computes x[m*128:(m+1)*128].
  - LSTM gates: column-sharded partial sums (core m contracts only its
    128-slice of x and of h0) + AllReduce over the [4096] partial gates;
    the elementwise LSTM cell is then computed replicated, so every core
    has the full h_new without another collective.
  - Output projection: vocab-on-partitions matvec in bf16 (halves the HBM
    traffic of the dominant 206MB weight), sharded logsumexp with a final
    AllReduce of per-partition exp-sums, log-softmax subtract on device.

The host side only slices / transposes / pads inputs into the per-core DMA
layouts, and reassembles the per-core output shards.
"""

import numpy as np
import ml_dtypes

H = 1024          # hidden size
L = 128           # attention length
V = 50257         # vocab
NCORES = 8
VS = 6400         # vocab shard per core (50 tiles of 128), 8*6400 = 51200
VT = VS // 128    # 50 vocab tiles per core
NEG_BIG = -1.0e30
NKC = H // 128    # 8 contraction chunks of 128 for H-sized dims

_BF16 = ml_dtypes.bfloat16
# attn/enc/comb/LSTM weights in bf16 (fp32 accumulate): saves ~10us of DMA
# but raises h_new/attn_weights error from ~1e-6 to ~3e-3 scale-relative.
# Kept off: the kernel already beats the memory-roofline target comfortably
# and the fp32 stage path keeps all non-logit outputs at fp32 exactness.
STAGE_BF16 = False
# Diagnostic only: replace collectives with local bounce copies (results
# become WRONG on multi-core; used to measure in-context collective cost).
NO_COLL = False

_prog_cache = {}


# ---------------------------------------------------------------------------
# device program
# ---------------------------------------------------------------------------

def _build_program(repeat=1):
    import concourse.bass as bass
    import concourse.mybir as mybir
    import concourse.tile as tile
    from concourse import bacc

    dt = mybir.dt
    f32 = dt.float32
    bf16 = dt.bfloat16
    sdt = bf16 if STAGE_BF16 else f32

    nc = bacc.Bacc("TRN2", target_bir_lowering=False, debug=False,
                   num_devices=NCORES)

    # ---- external inputs (per-core values supplied by the host) ----
    # emb shard has one extra all-zero row; non-owner cores gather that row
    # so the AllReduce-add broadcast needs no masking.
    emb_sh = nc.dram_tensor("emb_sh", [VS + 1, H], f32, kind="ExternalInput")
    gather_offs = nc.dram_tensor("gather_offs", [128, 1], dt.int32,
                                 kind="ExternalInput")
    enc = nc.dram_tensor("enc", [L, H], sdt, kind="ExternalInput")
    attn_we = nc.dram_tensor("attn_we", [128, NKC * 128], sdt,
                             kind="ExternalInput")
    attn_wh = nc.dram_tensor("attn_wh", [128, NKC * 128], sdt,
                             kind="ExternalInput")
    attn_b_col = nc.dram_tensor("attn_b_col", [128, 1], f32,
                                kind="ExternalInput")
    comb_we = nc.dram_tensor("comb_we", [128, NKC * 128], sdt,
                             kind="ExternalInput")
    comb_wa = nc.dram_tensor("comb_wa", [128, NKC * 128], sdt,
                             kind="ExternalInput")
    comb_b_col = nc.dram_tensor("comb_b_col", [128, 1], f32,
                                kind="ExternalInput")
    wih_sh = nc.dram_tensor("wih_sh", [128, 32 * 128], sdt,
                            kind="ExternalInput")
    whh_sh = nc.dram_tensor("whh_sh", [128, 32 * 128], sdt,
                            kind="ExternalInput")
    bias_g_col = nc.dram_tensor("bias_g_col", [128, 32], f32,
                                kind="ExternalInput")
    h0_col = nc.dram_tensor("h0_col", [128, NKC], sdt, kind="ExternalInput")
    c0_col = nc.dram_tensor("c0_col", [128, NKC], f32, kind="ExternalInput")
    h0_my = nc.dram_tensor("h0_my", [128, 1], sdt, kind="ExternalInput")
    # out-projection weights, bf16, 5 blocks of 10 vocab tiles each
    NBLK = 5
    TPB = VT // NBLK  # vocab tiles per block
    outw = [
        nc.dram_tensor(f"outw{b}", [128, TPB * 1024], bf16,
                       kind="ExternalInput")
        for b in range(NBLK)
    ]
    out_b_col = nc.dram_tensor("out_b_col", [128, VT], f32,
                               kind="ExternalInput")

    # ---- external outputs ----
    out_logprob = nc.dram_tensor("out_logprob", [128, VT], f32,
                                 kind="ExternalOutput")
    out_h = nc.dram_tensor("out_h", [128, NKC], f32, kind="ExternalOutput")
    out_c = nc.dram_tensor("out_c", [128, NKC], f32, kind="ExternalOutput")
    out_attw = nc.dram_tensor("out_attw", [128, 1], f32,
                              kind="ExternalOutput")

    RG = [list(range(NCORES))]

    with tile.TileContext(nc) as tc:
        with (
            tc.tile_pool(name="w", bufs=1) as wp,
            tc.tile_pool(name="sb", bufs=1) as sbp,
            tc.tile_pool(name="ps", bufs=2, space="PSUM") as psp,
            tc.tile_pool(name="dram", bufs=1, space="DRAM") as dram,
        ):
            for it in range(repeat):
                _build_body(nc, tc, wp, sbp, psp, dram, mybir, bass,
                            emb_sh, gather_offs, enc,
                            attn_we, attn_wh, attn_b_col,
                            comb_we, comb_wa, comb_b_col,
                            wih_sh, whh_sh, bias_g_col,
                            h0_col, c0_col, h0_my,
                            outw, out_b_col,
                            out_logprob, out_h, out_c, out_attw,
                            RG, NBLK, TPB, it)

    nc.compile()
    return nc


def _build_body(nc, tc, wp, sbp, psp, dram, mybir, bass,
                emb_sh, gather_offs, enc,
                attn_we, attn_wh, attn_b_col,
                comb_we, comb_wa, comb_b_col,
                wih_sh, whh_sh, bias_g_col,
                h0_col, c0_col, h0_my,
                outw, out_b_col,
                out_logprob, out_h, out_c, out_attw,
                RG, NBLK, TPB, it):
    dt = mybir.dt
    f32 = dt.float32
    bf16 = dt.bfloat16
    sdt = bf16 if STAGE_BF16 else f32
    AF = mybir.ActivationFunctionType
    OP = mybir.AluOpType

    # Tags are shared across repeat iterations: pool slots (and SBUF space)
    # are reused, and iteration i+1's loads serialize behind iteration i's
    # last reads, giving honest steady-state per-iteration timing.
    t = lambda name: name

    # ---- embedding gather + broadcast (AllReduce #1) -- issued FIRST so its
    # tiny DMAs and the collective are not queued behind the weight streams.
    # emb_sh [VS+1, H] viewed as [(VS+1)*128, 8]; row r = v*128 + q holds
    # emb_sh[v, q*8:(q+1)*8].  Gathered tile: g[p, j] = embedded[p*8 + j]
    # (a fixed permutation of the 1024 dims; the attn_we/comb_we host
    # layouts use the same permutation).  Non-owner cores gather the
    # all-zero row VS, so the AllReduce-add is an unmasked broadcast.
    offs_sb = wp.tile([128, 1], dt.int32, tag=t("offs"))
    nc.gpsimd.dma_start(offs_sb[:], gather_offs[:])
    emb_view = emb_sh.ap().rearrange("v (q e) -> (v q) e", e=8)
    gath = sbp.tile([128, 8], f32, tag=t("gath"))
    nc.gpsimd.indirect_dma_start(
        out=gath[:],
        out_offset=None,
        in_=emb_view,
        in_offset=bass.IndirectOffsetOnAxis(ap=offs_sb[:, :1], axis=0),
    )
    emb_bin = dram.tile([128, 8], f32, tag=t("emb_bin"))
    nc.gpsimd.dma_start(emb_bin[:], gath[:])
    emb_bout = dram.tile([128, 8], f32, tag=t("emb_bout"))
    if NO_COLL:
        nc.gpsimd.dma_start(emb_bout[:], emb_bin[:])
    else:
        nc.gpsimd.collective_compute(
            "AllReduce", OP.add, replica_groups=RG,
            ins=[emb_bin.opt()], outs=[emb_bout.opt()],
        )
    if STAGE_BF16:
        embP_f = sbp.tile([128, 8], f32, tag=t("embP_f"))
        nc.gpsimd.dma_start(embP_f[:], emb_bout[:])
        embP = sbp.tile([128, 8], sdt, tag=t("embP"))
        nc.vector.tensor_copy(embP[:], embP_f[:])
    else:
        embP = sbp.tile([128, 8], f32, tag=t("embP"))
        nc.gpsimd.dma_start(embP[:], emb_bout[:])

    # ---- load small tensors / weights into SBUF ----
    h0c_sb = wp.tile([128, NKC], sdt, tag=t("h0c"))
    nc.sync.dma_start(h0c_sb[:], h0_col[:])
    c0c_sb = wp.tile([128, NKC], f32, tag=t("c0c"))
    nc.sync.dma_start(c0c_sb[:], c0_col[:])
    h0my_sb = wp.tile([128, 1], sdt, tag=t("h0my"))
    nc.sync.dma_start(h0my_sb[:], h0_my[:])
    attnb_sb = wp.tile([128, 1], f32, tag=t("attnb"))
    nc.sync.dma_start(attnb_sb[:], attn_b_col[:])
    combb_sb = wp.tile([128, 1], f32, tag=t("combb"))
    nc.sync.dma_start(combb_sb[:], comb_b_col[:])
    biasg_sb = wp.tile([128, 32], f32, tag=t("biasg"))
    nc.sync.dma_start(biasg_sb[:], bias_g_col[:])
    outb_sb = wp.tile([128, VT], f32, tag=t("outb"))
    nc.sync.dma_start(outb_sb[:], out_b_col[:])

    awe_sb = wp.tile([128, NKC * 128], sdt, tag=t("awe"))
    nc.sync.dma_start(awe_sb[:], attn_we[:])
    awh_sb = wp.tile([128, NKC * 128], sdt, tag=t("awh"))
    nc.sync.dma_start(awh_sb[:], attn_wh[:])
    enc_sb = wp.tile([L, H], sdt, tag=t("enc"))
    nc.sync.dma_start(enc_sb[:], enc[:])
    cwe_sb = wp.tile([128, NKC * 128], sdt, tag=t("cwe"))
    nc.sync.dma_start(cwe_sb[:], comb_we[:])
    cwa_sb = wp.tile([128, NKC * 128], sdt, tag=t("cwa"))
    nc.sync.dma_start(cwa_sb[:], comb_wa[:])
    wih_sb = wp.tile([128, 32 * 128], sdt, tag=t("wih"))
    nc.sync.dma_start(wih_sb[:], wih_sh[:])
    whh_sb = wp.tile([128, 32 * 128], sdt, tag=t("whh"))
    nc.sync.dma_start(whh_sb[:], whh_sh[:])

    # outw block DMAs are emitted last among the loads: they are consumed
    # last (after h_new) and must not delay the small stage weights.
    outw_sb = []
    for b in range(NBLK):
        ow = wp.tile([128, TPB * 1024], bf16, tag=t(f"outw{b}"))
        nc.sync.dma_start(ow[:], outw[b][:])
        outw_sb.append(ow)

    # all-ones [128,128] matrix: matmul(ones_mat, v) puts sum_p(v[p]) on
    # every partition in one shot (sum + partition-broadcast combined)
    ones_mat = wp.tile([128, 128], f32, tag=t("ones_mat"))
    nc.vector.memset(ones_mat[:], 1.0)

    # ---- attention scores (all 128, replicated) ----
    ps_s = psp.tile([128, 1], f32, tag=t("ps_small"), bufs=2)
    for c in range(NKC):
        nc.tensor.matmul(ps_s[:], awh_sb[:, c * 128:(c + 1) * 128],
                         h0c_sb[:, c:c + 1], start=(c == 0), stop=False)
    for c in range(NKC):
        nc.tensor.matmul(ps_s[:], awe_sb[:, c * 128:(c + 1) * 128],
                         embP[:, c:c + 1], start=False, stop=(c == NKC - 1))
    # softmax over partitions; scores are O(1), exp without max-shift is safe
    exp_s = sbp.tile([128, 1], f32, tag=t("exp_s"))
    nc.scalar.activation(exp_s[:], ps_s[:], AF.Exp, bias=attnb_sb[:, :1])
    # sum over partitions, broadcast to every partition, in one matmul
    ps_sumb = psp.tile([128, 1], f32, tag=t("ps_small"), bufs=2)
    nc.tensor.matmul(ps_sumb[:], ones_mat[:], exp_s[:], start=True, stop=True)
    rinv_col = sbp.tile([128, 1], f32, tag=t("rinv_col"))
    nc.vector.reciprocal(rinv_col[:], ps_sumb[:])
    attw = sbp.tile([128, 1], f32, tag=t("attw"))
    nc.vector.tensor_mul(attw[:], exp_s[:], rinv_col[:])
    nc.scalar.dma_start(out_attw[:], attw[:])
    if STAGE_BF16:
        attw_s = sbp.tile([128, 1], sdt, tag=t("attw_s"))
        nc.vector.tensor_copy(attw_s[:], attw[:])
    else:
        attw_s = attw

    # ---- attn_applied, directly in column-chunk form [128, 8] ----
    ps_aa = psp.tile([128, 8], f32, tag=t("ps_small"), bufs=2)
    for c in range(NKC):
        nc.tensor.matmul(ps_aa[:, c:c + 1], enc_sb[:, c * 128:(c + 1) * 128],
                         attw_s[:], start=True, stop=True)
    aa_sb = sbp.tile([128, 8], sdt, tag=t("aa"))
    nc.vector.tensor_copy(aa_sb[:], ps_aa[:])

    # ---- combine + relu: x slice for this core ----
    ps_x = psp.tile([128, 1], f32, tag=t("ps_small"), bufs=2)
    for c in range(NKC):
        nc.tensor.matmul(ps_x[:], cwe_sb[:, c * 128:(c + 1) * 128],
                         embP[:, c:c + 1], start=(c == 0), stop=False)
    for c in range(NKC):
        nc.tensor.matmul(ps_x[:], cwa_sb[:, c * 128:(c + 1) * 128],
                         aa_sb[:, c:c + 1], start=False, stop=(c == NKC - 1))
    x_col = sbp.tile([128, 1], sdt, tag=t("x_col"))
    nc.scalar.activation(x_col[:], ps_x[:], AF.Relu, bias=combb_sb[:, :1])

    # ---- partial LSTM gates (column-sharded) + AllReduce #2 ----
    # W_hh @ h0 into its own PSUM bank first: it depends only on inputs, so
    # the PE retires half the gate matmuls (and DVE parks them in SBUF)
    # while the embedded broadcast / attention / combine chain is still in
    # flight; the x-dependent half lands in a second bank and one DVE add
    # replaces the old psum->sbuf copy on the critical path.
    ps_gh = psp.tile([128, 32], f32, tag=t("ps_gh"), bufs=1)
    for tau in range(32):
        sl = slice(tau * 128, (tau + 1) * 128)
        nc.tensor.matmul(ps_gh[:, tau:tau + 1], whh_sb[:, sl], h0my_sb[:],
                         start=True, stop=True)
    gh_sb = sbp.tile([128, 32], f32, tag=t("gh"))
    nc.vector.tensor_copy(gh_sb[:], ps_gh[:])
    ps_gi = psp.tile([128, 32], f32, tag=t("ps_gi"), bufs=1)
    for tau in range(32):
        sl = slice(tau * 128, (tau + 1) * 128)
        nc.tensor.matmul(ps_gi[:, tau:tau + 1], wih_sb[:, sl], x_col[:],
                         start=True, stop=True)
    gpart = sbp.tile([128, 32], f32, tag=t("gpart"))
    nc.vector.tensor_add(gpart[:], gh_sb[:], ps_gi[:])
    g_bin = dram.tile([128, 32], f32, tag=t("g_bin"))
    nc.gpsimd.dma_start(g_bin[:], gpart[:])
    g_bout = dram.tile([128, 32], f32, tag=t("g_bout"))
    if NO_COLL:
        nc.gpsimd.dma_start(g_bout[:], g_bin[:])
    else:
        nc.gpsimd.collective_compute(
            "AllReduce", OP.add, replica_groups=RG,
            ins=[g_bin.opt()], outs=[g_bout.opt()],
        )
    gsum = sbp.tile([128, 32], f32, tag=t("gsum"))
    nc.gpsimd.dma_start(gsum[:], g_bout[:])

    # ---- LSTM cell elementwise (replicated) ----
    gf = sbp.tile([128, 32], f32, tag=t("gf"))
    nc.vector.tensor_add(gf[:], gsum[:], biasg_sb[:])
    sig_i = sbp.tile([128, 8], f32, tag=t("sig_i"))
    nc.scalar.activation(sig_i[:], gf[:, 0:8], AF.Sigmoid)
    sig_f = sbp.tile([128, 8], f32, tag=t("sig_f"))
    nc.scalar.activation(sig_f[:], gf[:, 8:16], AF.Sigmoid)
    tanh_g = sbp.tile([128, 8], f32, tag=t("tanh_g"))
    nc.scalar.activation(tanh_g[:], gf[:, 16:24], AF.Tanh)
    sig_o = sbp.tile([128, 8], f32, tag=t("sig_o"))
    nc.scalar.activation(sig_o[:], gf[:, 24:32], AF.Sigmoid)
    t1 = sbp.tile([128, 8], f32, tag=t("t1"))
    nc.vector.tensor_mul(t1[:], sig_f[:], c0c_sb[:])
    t2 = sbp.tile([128, 8], f32, tag=t("t2"))
    nc.vector.tensor_mul(t2[:], sig_i[:], tanh_g[:])
    c_new = sbp.tile([128, 8], f32, tag=t("c_new"))
    nc.vector.tensor_add(c_new[:], t1[:], t2[:])
    tanh_c = sbp.tile([128, 8], f32, tag=t("tanh_c"))
    nc.scalar.activation(tanh_c[:], c_new[:], AF.Tanh)
    h_new = sbp.tile([128, 8], f32, tag=t("h_new"))
    nc.vector.tensor_mul(h_new[:], sig_o[:], tanh_c[:])
    nc.scalar.dma_start(out_h[:], h_new[:])
    nc.scalar.dma_start(out_c[:], c_new[:])
    h_bf = sbp.tile([128, 8], bf16, tag=t("h_bf"))
    nc.vector.tensor_copy(h_bf[:], h_new[:])

    # ---- output projection matvec (vocab on partitions, bf16) ----
    logits = sbp.tile([128, VT], f32, tag=t("logits"))
    for v in range(VT):
        blk, off = divmod(v, TPB)
        base = off * 1024
        po = psp.tile([128, 1], f32, tag=t("po"), bufs=4)
        for c in range(NKC):
            nc.tensor.matmul(po[:], outw_sb[blk][:, base + c * 128:
                                                 base + (c + 1) * 128],
                             h_bf[:, c:c + 1],
                             start=(c == 0), stop=(c == NKC - 1))
        nc.vector.tensor_add(logits[:, v:v + 1], po[:], outb_sb[:, v:v + 1])

    # ---- sharded logsumexp (AllReduce #3) + log-softmax ----
    # exp and its per-partition row-sum fused in one ACT op via accum_out
    exps = sbp.tile([128, VT], f32, tag=t("exps"))
    se_col = sbp.tile([128, 1], f32, tag=t("se_col"))
    nc.scalar.activation(exps[:], logits[:], AF.Exp, accum_out=se_col[:])
    se_bin = dram.tile([128, 1], f32, tag=t("se_bin"))
    nc.gpsimd.dma_start(se_bin[:], se_col[:])
    se_bout = dram.tile([128, 1], f32, tag=t("se_bout"))
    if NO_COLL:
        nc.gpsimd.dma_start(se_bout[:], se_bin[:])
    else:
        nc.gpsimd.collective_compute(
            "AllReduce", OP.add, replica_groups=RG,
            ins=[se_bin.opt()], outs=[se_bout.opt()],
        )
    se_g = sbp.tile([128, 1], f32, tag=t("se_g"))
    nc.gpsimd.dma_start(se_g[:], se_bout[:])
    # grand total on every partition in one matmul, then logZ and subtract
    ps_totb = psp.tile([128, 1], f32, tag=t("ps_small"), bufs=2)
    nc.tensor.matmul(ps_totb[:], ones_mat[:], se_g[:], start=True, stop=True)
    lnzb = sbp.tile([128, 1], f32, tag=t("lnzb"))
    nc.scalar.activation(lnzb[:], ps_totb[:], AF.Ln)
    out_lp = sbp.tile([128, VT], f32, tag=t("out_lp"))
    nc.vector.tensor_tensor(out=out_lp[:], in0=logits[:],
                            in1=lnzb[:, :1].to_broadcast([128, VT]),
                            op=OP.subtract)
    nc.scalar.dma_start(out_logprob[:], out_lp[:])


# ---------------------------------------------------------------------------
# host-side sharding
# ---------------------------------------------------------------------------

def _shard_inputs(tokens, h0, c0, encoder_outputs, emb, attn_W, attn_b,
                  comb_W, comb_b, W_ih, W_hh, b_ih, b_hh, out_W, out_b):
    f32 = np.float32
    tokens = np.asarray(tokens)
    h0 = np.asarray(h0, f32).reshape(H)
    c0 = np.asarray(c0, f32).reshape(H)
    enc = np.ascontiguousarray(np.asarray(encoder_outputs, f32))
    emb = np.asarray(emb, f32)
    attn_W = np.asarray(attn_W, f32)
    attn_b = np.asarray(attn_b, f32).reshape(L)
    comb_W = np.asarray(comb_W, f32)
    comb_b = np.asarray(comb_b, f32).reshape(H)
    W_ih = np.asarray(W_ih, f32)
    W_hh = np.asarray(W_hh, f32)
    b_ih = np.asarray(b_ih, f32).reshape(4 * H)
    b_hh = np.asarray(b_hh, f32).reshape(4 * H)
    out_W = np.asarray(out_W, f32)
    out_b = np.asarray(out_b, f32).reshape(V)

    tok = int(np.asarray(tokens).reshape(-1)[0])
    owner = tok // VS
    loc = tok - owner * VS

    # replicated pieces
    # attn_we[p, c*128+l] = attn_W[l, p*8+c]  (permuted embedded index p*8+c)
    A = attn_W[:, :H].reshape(L, 128, 8)            # [l][p][c]
    attn_we = np.ascontiguousarray(
        A.transpose(1, 2, 0).reshape(128, NKC * 128))
    B = attn_W[:, H:].reshape(L, 8, 128)            # [l][c][p]
    attn_wh = np.ascontiguousarray(
        B.transpose(2, 1, 0).reshape(128, NKC * 128))
    attn_b_col = np.ascontiguousarray(attn_b.reshape(128, 1))
    bias_g_col = np.ascontiguousarray((b_ih + b_hh).reshape(32, 128).T)
    h0_col = np.ascontiguousarray(h0.reshape(8, 128).T)
    c0_col = np.ascontiguousarray(c0.reshape(8, 128).T)

    # padded vocab-sharded tables
    emb_pad = np.zeros((NCORES * VS, H), f32)
    emb_pad[:V] = emb
    outw_pad = np.zeros((NCORES * VS, H), f32)
    outw_pad[:V] = out_W
    outb_pad = np.full(NCORES * VS, NEG_BIG, f32)
    outb_pad[:V] = out_b

    in_maps = []
    for m in range(NCORES):
        r0 = m * VS
        # comb: rows for this core's x slice
        C = comb_W[m * 128:(m + 1) * 128, :H].reshape(128, 128, 8)  # [j][p][c]
        comb_we = np.ascontiguousarray(
            C.transpose(1, 2, 0).reshape(128, NKC * 128))
        D = comb_W[m * 128:(m + 1) * 128, H:].reshape(128, 8, 128)  # [j][c][p]
        comb_wa = np.ascontiguousarray(
            D.transpose(2, 1, 0).reshape(128, NKC * 128))
        comb_b_col = np.ascontiguousarray(
            comb_b[m * 128:(m + 1) * 128].reshape(128, 1))
        # LSTM column shards: [k][tau*128+j] = W[tau*128+j, m*128+k]
        wih_sh = np.ascontiguousarray(W_ih[:, m * 128:(m + 1) * 128].T)
        whh_sh = np.ascontiguousarray(W_hh[:, m * 128:(m + 1) * 128].T)
        h0_my = np.ascontiguousarray(h0_col[:, m:m + 1])
        # out projection: outw[p, t*1024 + c*128 + j] = W[r0+t*128+j, c*128+p]
        S = outw_pad[r0:r0 + VS].reshape(VT, 128, 8, 128)  # [t][j][c][p]
        ow = np.ascontiguousarray(
            S.transpose(3, 0, 2, 1).reshape(128, VT * 1024)).astype(_BF16)
        out_b_col = np.ascontiguousarray(
            outb_pad[r0:r0 + VS].reshape(VT, 128).T)
        # embedding shard (+1 zero row) + gather offsets
        emb_shard = np.zeros((VS + 1, H), f32)
        emb_shard[:VS] = emb_pad[r0:r0 + VS]
        if m == owner:
            offs = (loc * 128 + np.arange(128, dtype=np.int32))
        else:
            offs = (VS * 128 + np.arange(128, dtype=np.int32))
        snp = _BF16 if STAGE_BF16 else f32
        im = {
            "emb_sh": emb_shard,
            "gather_offs": offs.reshape(128, 1),
            "enc": enc.astype(snp),
            "attn_we": attn_we.astype(snp),
            "attn_wh": attn_wh.astype(snp),
            "attn_b_col": attn_b_col,
            "comb_we": comb_we.astype(snp),
            "comb_wa": comb_wa.astype(snp),
            "comb_b_col": comb_b_col,
            "wih_sh": wih_sh.astype(snp),
            "whh_sh": whh_sh.astype(snp),
            "bias_g_col": bias_g_col,
            "h0_col": h0_col.astype(snp),
            "c0_col": c0_col,
            "h0_my": h0_my.astype(snp),
            "out_b_col": out_b_col,
        }
        NBLK = 5
        TPB = VT // NBLK
        for b in range(NBLK):
            im[f"outw{b}"] = np.ascontiguousarray(
                ow[:, b * TPB * 1024:(b + 1) * TPB * 1024])
        in_maps.append(im)
    return in_maps


def _assemble(results):
    logp = np.concatenate(
        [results[m]["out_logprob"].T.reshape(VS) for m in range(NCORES)])
    output = logp[:V].reshape(1, V).astype(np.float32)
    h_new = results[0]["out_h"].T.reshape(1, 1, H).astype(np.float32)
    c_new = results[0]["out_c"].T.reshape(1, 1, H).astype(np.float32)
    attw = results[0]["out_attw"].reshape(1, L).astype(np.float32)
    return output, h_new, c_new, attw


# ---------------------------------------------------------------------------
# public entry point
# ---------------------------------------------------------------------------

def _get_program(repeat=1):
    if repeat not in _prog_cache:
        _prog_cache[repeat] = _build_program(repeat)
    return _prog_cache[repeat]


_runner_cache = {}


def _make_runner(nc):
    """Persistent jitted runner for repeat calls (avoids re-jitting the
    NEFF custom call inside run_bass_kernel_spmd on every invocation)."""
    import jax
    from jax.sharding import Mesh, PartitionSpec, NamedSharding
    from jax.experimental.shard_map import shard_map
    from concourse import bass2jax, mybir

    bass2jax.install_neuronx_cc_hook()
    partition_name = (nc.partition_id_tensor.name
                      if nc.partition_id_tensor else None)
    in_names, out_names, out_avals, zero_shapes = [], [], [], []
    for alloc in nc.m.functions[0].allocations:
        if not isinstance(alloc, mybir.MemoryLocationSet):
            continue
        name = alloc.memorylocations[0].name
        if alloc.kind == "ExternalInput":
            if name != partition_name:
                in_names.append(name)
        elif alloc.kind == "ExternalOutput":
            out_names.append(name)
            shape = tuple(alloc.tensor_shape)
            dtype = mybir.dt.np(alloc.dtype)
            out_avals.append(jax.core.ShapedArray(shape, dtype))
            zero_shapes.append((shape, dtype))
    n_params = len(in_names)
    n_outs = len(out_names)
    all_names = in_names + out_names
    if partition_name is not None:
        all_names = all_names + [partition_name]

    def _body(*args):
        operands = list(args)
        if partition_name is not None:
            operands.append(bass2jax.partition_id_tensor())
        outs = bass2jax._bass_exec_p.bind(
            *operands,
            out_avals=tuple(out_avals),
            in_names=tuple(all_names),
            out_names=tuple(out_names),
            lowering_input_output_aliases=(),
            sim_require_finite=True,
            sim_require_nnan=True,
            nc=nc,
        )
        return tuple(outs)

    devices = jax.devices()[:NCORES]
    mesh = Mesh(np.asarray(devices), ("core",))
    in_specs = (PartitionSpec("core"),) * (n_params + n_outs)
    out_specs = (PartitionSpec("core"),) * n_outs
    donate = tuple(range(n_params, n_params + n_outs))
    fn = jax.jit(
        shard_map(_body, mesh=mesh, in_specs=in_specs, out_specs=out_specs,
                  check_rep=False),
        donate_argnums=donate, keep_unused=True,
    )
    sharding = NamedSharding(mesh, PartitionSpec("core"))

    def run(in_maps):
        concat_in = [
            jax.device_put(
                np.concatenate([np.asarray(in_maps[c][n])
                                for c in range(NCORES)], axis=0), sharding)
            for n in in_names
        ]
        zeros = [
            jax.device_put(np.zeros((NCORES * s[0], *s[1:]), d), sharding)
            for s, d in zero_shapes
        ]
        outs = fn(*concat_in, *zeros)
        jax.block_until_ready(outs)
        return [
            {name: np.asarray(outs[i]).reshape(NCORES, *out_avals[i].shape)[c]
             for i, name in enumerate(out_names)}
            for c in range(NCORES)
        ]

    return run


def kernel(**inputs):
    from concourse.bass_utils import run_bass_kernel_spmd

    nc = _get_program()
    in_maps = _shard_inputs(**inputs)
    if "run" not in _runner_cache:
        # first call: the sanctioned SPMD path (compiles the NEFF)
        res = run_bass_kernel_spmd(nc, in_maps, list(range(NCORES)))
        _runner_cache["run"] = _make_runner(nc)
        return _assemble(res.results)
    return _assemble(_runner_cache["run"](in_maps))
